# revision 53
# baseline (speedup 1.0000x reference)
"""Trainium2 Bass kernel for a 2-layer GAT (nn_GAT_197568496078).

Strategy (8 NeuronCores, SPMD single program):
  - Edges (+self loops) are sharded by DESTINATION node range: core c owns
    dst in [c*6250, (c+1)*6250). Aggregation is then core-local (no
    collectives). The node feature table is replicated (each core builds it
    with small matmuls; the build DMAs are batched 8 iterations per
    dma_start, since the per-dma_start issue cost on the sync sequencer
    dominated the table-build prologue).
  - Per layer, each core builds an HBM table T1[n] = [h(128) | alpha_src(2)]
    (fp16, 512B rows) and a small per-shard table T2[v_local] = alpha_dst(2),
    then streams its edges one dst-WINDOW (127 dst nodes) at a time:
      gather T1 rows by src (dma_gather; two src halves so indices fit
      int16, each half split in two and spread over 4 SWDGE queues: each
      queue has its own Q7 cpu pair, so descriptor generation -- the
      original bottleneck at ~6ns/row on a single queue -- runs 4-way
      parallel, and each queue's private descriptor ring stays under its
      128-descriptor capacity),
      one-hot S[e, t, j] = (iota_j == dst_rel)  (bf16, DVE),
      transposed one-hot SgT[j, t, e] = (j == dst_rel)  (fp16, DVE, from a
        partition-replicated row-major dst_rel loaded by 0-stride DMA),
      per-edge alpha_dst WITHOUT a per-edge gather: ad[e, h] = SgT_t.T @
        adwin on TensorE (adwin = the window's 127 alpha_dst rows,
        partition-major, one tiny contiguous DMA),
      score = leaky_relu(as + ad); w = exp(score)  (bf16 has the range for
        scores in [-16, 16]; the softmax max-shift cancels exactly in the
        reference, so it is skipped),
      M = [h * w | w]  (bf16),
      psum[j, 0:130] += S.T @ M   (TensorE, fp32 PSUM, per 127-dst window)
    flush: out[j] = msg/denom (+bias, relu/mean-heads).
  - Two launches (layer 1, layer 2); the host re-shards layer-1 output
    between them (index-only work).
  - Per-window tile counts are padded to the max over cores so all 8 cores
    run one identical program; all per-core variation lives in data arrays
    (gather indices, dst_rel).
  - 10.3ms -> 2.4-2.6ms total HW time vs the single-queue/per-edge-T2-gather
    baseline; rel err vs the fp32 reference ~1e-3.
"""
import os
import sys
import numpy as np
import ml_dtypes

sys.path.insert(0, "/opt/trn_rl_repo")

import concourse.bacc as bacc   # noqa: E402
import concourse.bass as bass   # noqa: E402
import concourse.mybir as mybir # noqa: E402
import concourse.tile as tile   # noqa: E402
from concourse.alu_op_type import AluOpType          # noqa: E402
from concourse.bass_utils import run_bass_kernel_spmd  # noqa: E402
from concourse.library_config import mlp             # noqa: E402

bf16 = ml_dtypes.bfloat16
f16 = np.float16
dt = mybir.dt
AF = mybir.ActivationFunctionType

N, IN_DIM, HID, HEADS, OUT_DIM, E = 50000, 128, 64, 2, 64, 1600000
NCORES = 8
NPC = N // NCORES            # 6250
WIN = 127                    # dst nodes per window (col 127 = pad trash)
NWIN = -(-NPC // WIN)        # 50
TILE = 128
HALF_LIM = 25600             # src < HALF_LIM -> half A (idx base 0); aligned
BASE_B = HALF_LIM            # to the 8-block build batches so the A table is
NROWS_A = HALF_LIM           # a whole number of batched build chunks
NTAB = 391 * 128             # 50048 table rows (N padded to 128)
NROWS_B = NTAB - HALF_LIM    # 24448; half B idx = src - BASE_B
NSH = 50 * 128               # 6400 shard rows (>= NWIN*WIN = 6350)
OUT_ROWS = NWIN * WIN        # 6350

# module-level memo: preprocessing + compiled programs are reused across calls
_CACHE = {}
LAST_EXEC_NS = []            # exec_time_ns of the launches from the last call
LAST_RESULTS = []            # full BassKernelResults of the last call (trace mode)


def _register_ntff_hook():
    """Provide antenv.axon_hooks (absent in this container) so
    run_bass_kernel_spmd(trace=True) can capture NTFF profiles."""
    import types
    import ctypes
    import contextlib

    if "antenv.axon_hooks" in sys.modules:
        return
    try:
        lib = ctypes.CDLL("/opt/axon/libaxon_pjrt.so")
        lib.axon_start_nrt_profile.argtypes = [
            ctypes.POINTER(ctypes.c_int64), ctypes.c_size_t]
        lib.axon_start_nrt_profile.restype = ctypes.c_int64
        lib.axon_stop_nrt_profile.argtypes = [ctypes.c_char_p]
        lib.axon_stop_nrt_profile.restype = ctypes.c_int64
    except (OSError, AttributeError):
        return

    @contextlib.contextmanager
    def _hook(output_dir, device_ids):
        import jax
        jax.devices()
        if device_ids:
            ids = (ctypes.c_int64 * len(device_ids))(*device_ids)
            rc = lib.axon_start_nrt_profile(ids, len(device_ids))
        else:
            rc = lib.axon_start_nrt_profile(None, 0)
        if rc != 0:
            raise RuntimeError(f"axon_start_nrt_profile rc={rc}")
        try:
            yield
        finally:
            n = lib.axon_stop_nrt_profile(str(output_dir).encode())
            print(f"ntff profile: {n} file(s) -> {output_dir}", file=sys.stderr)

    mod = types.ModuleType("antenv.axon_hooks")
    mod.get_axon_ntff_profile_hook = lambda: _hook
    sys.modules["antenv.axon_hooks"] = mod
    # avoid network uploads during offline trace processing
    import concourse.bass_utils as _bu
    _bu.upload_artifacts = lambda p: str(p)


# --------------------------------------------------------------------------
# host-side graph preprocessing (index-only)
# --------------------------------------------------------------------------

def _schedule(edge_index):
    src = np.concatenate([edge_index[0], np.arange(N)]).astype(np.int64)
    dst = np.concatenate([edge_index[1], np.arange(N)]).astype(np.int64)
    shard = dst // NPC

    # collect per (core, window, half) edge lists
    per = [[None] * NWIN for _ in range(NCORES)]
    for c in range(NCORES):
        m = shard == c
        s, d = src[m], dst[m] - c * NPC
        wi = d // WIN
        for w in range(NWIN):
            wm = wi == w
            ws, wd = s[wm], d[wm] - w * WIN
            a = ws < HALF_LIM
            per[c][w] = ((ws[a], wd[a]), (ws[~a], wd[~a]))

    # uniform tile counts per (window, half) = max over cores
    nA = [max(-(-len(per[c][w][0][0]) // TILE) for c in range(NCORES))
          for w in range(NWIN)]
    nB = [max(-(-len(per[c][w][1][0]) // TILE) for c in range(NCORES))
          for w in range(NWIN)]
    ntot = sum(nA) + sum(nB)

    t1 = np.zeros((NCORES, ntot * TILE), np.int16)
    dr = np.zeros((NCORES, ntot * TILE), np.float32)
    for c in range(NCORES):
        pos = 0
        for w in range(NWIN):
            for half, ntiles in ((0, nA[w]), (1, nB[w])):
                ws, wd = per[c][w][half]
                ne, cap = len(ws), ntiles * TILE
                pad = cap - ne
                base = 0 if half == 0 else BASE_B
                # pad idx = -1: the Q7 ucode trims trailing negative indices,
                # so each core only generates descriptors for its true edges
                if os.environ.get("K_TRIM", "0") == "1":
                    fs = np.concatenate([ws - base, np.full(pad, -1)])
                else:
                    psrc = np.full(pad, ws[-1] if ne else base)
                    fs = np.concatenate([ws, psrc]) - base
                fd = np.concatenate([wd, np.full(pad, WIN)])
                t1[c, pos:pos + cap] = fs.astype(np.int16)
                dr[c, pos:pos + cap] = fd
                pos += cap
        assert pos == ntot * TILE

    def wrap_idx(a):  # -> [128, n/16] wrapped for the 8 Q7 cores
        return np.ascontiguousarray(np.tile(a.reshape(-1, 16).T, (8, 1)))

    i1 = [wrap_idx(t1[c]) for c in range(NCORES)]
    drel = [np.ascontiguousarray(dr[c].reshape(-1, TILE).T.astype(bf16))
            for c in range(NCORES)]
    drel_rm = [np.ascontiguousarray(dr[c].reshape(-1, TILE).astype(f16))
               for c in range(NCORES)]
    return {"nA": nA, "nB": nB, "ntot": ntot, "i1": i1, "drel": drel,
            "drel_rm": drel_rm}


def _expand_att(a):
    """att [heads, dim] -> [128, heads] block-diagonal expansion (layout only)."""
    heads, dim = a.shape
    out = np.zeros((heads * dim, heads), np.float32)
    for h in range(heads):
        out[h * dim:(h + 1) * dim, h] = a[h]
    return out.astype(f16)


# --------------------------------------------------------------------------
# device program (identical for all cores; layer 1/2 differ only in flush)
# --------------------------------------------------------------------------

def _build_program(layer, sched, nwin=NWIN):
    nA, nB, ntot = sched["nA"], sched["nB"], sched["ntot"]
    GW = max(nA[w] + nB[w] for w in range(nwin))
    NSWQ = int(os.environ.get("K_NSWQ", "4"))
    nc = bacc.Bacc("TRN2", target_bir_lowering=False, debug=False,
                   enable_asserts=False, num_devices=NCORES,
                   num_swdge_queues=NSWQ)

    xT = nc.dram_tensor("xT", [128, NTAB], dt.float16, kind="ExternalInput")
    xTs = nc.dram_tensor("xTs", [128, NSH], dt.float16, kind="ExternalInput")
    W = nc.dram_tensor("W", [128, 128], dt.float16, kind="ExternalInput")
    WT = nc.dram_tensor("WT", [128, 128], dt.float16, kind="ExternalInput")
    Ase = nc.dram_tensor("Ase", [128, 2], dt.float16, kind="ExternalInput")
    Ade = nc.dram_tensor("Ade", [128, 2], dt.float16, kind="ExternalInput")
    brep = nc.dram_tensor("brep", [128, 128], dt.float32, kind="ExternalInput")
    iota = nc.dram_tensor("iota", [128, 128], dt.bfloat16, kind="ExternalInput")
    iotaP = nc.dram_tensor("iotaP", [128, 1], dt.float16, kind="ExternalInput")
    i1d = nc.dram_tensor("i1", [128, ntot * 8], dt.int16, kind="ExternalInput")
    dreld = nc.dram_tensor("drel", [128, ntot], dt.bfloat16, kind="ExternalInput")
    drelrmd = nc.dram_tensor("drel_rm", [ntot, TILE], dt.float16,
                             kind="ExternalInput")
    if layer == 1:
        outd = nc.dram_tensor("out", [OUT_ROWS, 128], dt.float16,
                              kind="ExternalOutput")
    else:
        outd = nc.dram_tensor("out", [OUT_ROWS, 64], dt.float32,
                              kind="ExternalOutput")

    with tile.TileContext(nc) as tc:
        with (
            tc.tile_pool(name="const", bufs=1) as constp,
            tc.tile_pool(name="tb", bufs=3) as tbp,
            tc.tile_pool(name="t1p", bufs=3) as t1p,
            tc.tile_pool(name="ohp", bufs=3 if layer == 1 else 2) as ohp,
            tc.tile_pool(name="work", bufs=2) as work,
            tc.tile_pool(name="adp", bufs=3) as adp,
            tc.tile_pool(name="fl", bufs=2) as flp,
            tc.tile_pool(name="pst", bufs=2, space="PSUM") as pst,
            tc.tile_pool(name="psw", bufs=2, space="PSUM") as psw,
            tc.tile_pool(name="psa", bufs=2, space="PSUM") as psa,
            tc.tile_pool(name="dram", bufs=1, space="DRAM") as dram,
        ):
            nc.gpsimd.load_library(mlp)

            # two separate DRAM tensors for the two src halves: the tile
            # framework tracks dependencies per tensor, so half-A gathers can
            # start while half B of the table is still being built
            T1A_dram = dram.tile([NROWS_A, 256], dt.float16)
            T1B_dram = dram.tile([NROWS_B, 256], dt.float16)
            T2_dram = dram.tile([NSH, 2], dt.float16)

            # ---- constants
            iota_sb = constp.tile([128, 128], dt.bfloat16)
            nc.sync.dma_start(iota_sb[:], iota[:])
            iotap_sb = constp.tile([128, 1], dt.float16)
            nc.sync.dma_start(iotap_sb[:], iotaP[:])
            brep_sb = constp.tile([128, 128], dt.float32)
            nc.sync.dma_start(brep_sb[:], brep[:])
            i1_sb = constp.tile([128, ntot * 8], dt.int16)
            nc.sync.dma_start(i1_sb[:], i1d[:])
            drel_sb = constp.tile([128, ntot], dt.bfloat16)
            nc.sync.dma_start(drel_sb[:], dreld[:])

            # ---- weight fold: We = [W | W @ Ase]; wd = W @ Ade
            wt_sb = constp.tile([128, 128], dt.float16)
            nc.sync.dma_start(wt_sb[:], WT[:])
            ase_sb = constp.tile([128, 2], dt.float16)
            nc.sync.dma_start(ase_sb[:], Ase[:])
            ade_sb = constp.tile([128, 2], dt.float16)
            nc.sync.dma_start(ade_sb[:], Ade[:])
            we_sb = constp.tile([128, 130], dt.float16)
            nc.sync.dma_start(we_sb[:, 0:128], W[:])
            wd_sb = constp.tile([128, 2], dt.float16)
            ps = pst.tile([128, 2], dt.float32, tag="pt")
            nc.tensor.matmul(ps[:], wt_sb[:], ase_sb[:])
            nc.scalar.activation(out=we_sb[:, 128:130], in_=ps[:], func=AF.Copy)
            ps2 = pst.tile([128, 2], dt.float32, tag="pt")
            nc.tensor.matmul(ps2[:], wt_sb[:], ade_sb[:])
            nc.scalar.activation(out=wd_sb[:], in_=ps2[:], func=AF.Copy)

            # ---- T2 table build first (windows' alpha_dst depends on it).
            # Batched like the T1 build: one in/out dma_start per 10 blocks
            # (the sync-sequencer issue cost, not bandwidth, is what counts).
            B2 = 10
            for j0 in range(0, NSH // 128, B2):
                nb = min(B2, NSH // 128 - j0)
                xs = tbp.tile([128, B2 * 128], dt.float16, tag="xs")
                nc.sync.dma_start(xs[:, 0:nb * 128],
                                  xTs[:, j0 * 128:(j0 + nb) * 128])
                t2b = tbp.tile([128, B2, 2], dt.float16, tag="t2out")
                for k in range(nb):
                    p2 = pst.tile([128, 2], dt.float32, tag="pt2")
                    nc.tensor.matmul(p2[:], xs[:, k * 128:(k + 1) * 128],
                                     wd_sb[:])
                    nc.scalar.activation(out=t2b[:, k, :], in_=p2[:],
                                         func=AF.Copy)
                d2 = T2_dram[j0 * 128:(j0 + nb) * 128, :]
                d2ap = bass.AP(tensor=d2.tensor, offset=d2.offset,
                               ap=[[2, 128], [128 * 2, nb], [1, 2]])
                nc.sync.dma_start(d2ap, t2b[:, 0:nb, :])

            # ---- T1 table build: [h | alpha_src] for all N.
            # DMAs batched 8 iterations at a time: the per-dma_start issue
            # cost on the sync sequencer (~0.7us) dominated the build.
            # Half A is built first so its gathers overlap half B's build.
            B1 = 8

            def build_t1(dst_tile, col0, nrows):
                for j0 in range(0, nrows // 128, B1):
                    nb = min(B1, nrows // 128 - j0)
                    xt = tbp.tile([128, B1 * 128], dt.float16, tag="xt")
                    nc.sync.dma_start(
                        xt[:, 0:nb * 128],
                        xT[:, col0 + j0 * 128:col0 + (j0 + nb) * 128])
                    tb = tbp.tile([128, B1, 130], dt.float16, tag="tbout")
                    for k in range(nb):
                        pt = pst.tile([128, 130], dt.float32, tag="pt")
                        nc.tensor.matmul(pt[:], xt[:, k * 128:(k + 1) * 128],
                                         we_sb[:])
                        nc.scalar.activation(out=tb[:, k, :], in_=pt[:],
                                             func=AF.Copy)
                    dview = dst_tile[j0 * 128:(j0 + nb) * 128, 0:130]
                    dap = bass.AP(tensor=dview.tensor, offset=dview.offset,
                                  ap=[[256, 128], [128 * 256, nb], [1, 130]])
                    nc.sync.dma_start(dap, tb[:, 0:nb, :])

            build_t1(T1A_dram, 0, NROWS_A)
            build_t1(T1B_dram, NROWS_A, NROWS_B)

            # ---- edge pipeline
            t1A = T1A_dram[:]
            t1B = T1B_dram[:]
            def flush_window(w, pw):
                # denom >= exp(LR(-16)) ~ 0.04 for real rows (self loop);
                # +1e-6 keeps the trash/pad rows away from reciprocal(0).
                rd = flp.tile([128, 2], dt.float32, tag="rd")
                nc.vector.tensor_scalar(
                    out=rd[:], in0=pw[:, 128:130], scalar1=1e-6, scalar2=None,
                    op0=AluOpType.add)
                r = flp.tile([128, 2], dt.float32, tag="r")
                nc.vector.reciprocal(r[:], rd[:])
                if layer == 1:
                    f32t = flp.tile([128, 128], dt.float32, tag="f32")
                    for h in range(HEADS):
                        nc.vector.scalar_tensor_tensor(
                            out=f32t[:, h * 64:(h + 1) * 64],
                            in0=pw[:, h * 64:(h + 1) * 64],
                            scalar=r[:, h:h + 1],
                            in1=brep_sb[:, h * 64:(h + 1) * 64],
                            op0=AluOpType.mult, op1=AluOpType.add)
                    ob = flp.tile([128, 128], dt.float16, tag="ob")
                    nc.scalar.activation(out=ob[:], in_=f32t[:], func=AF.Relu)
                    nc.sync.dma_start(outd[w * WIN:(w + 1) * WIN, :],
                                      ob[0:WIN, :])
                else:
                    ta = flp.tile([128, 64], dt.float32, tag="ta")
                    nc.vector.tensor_scalar(
                        out=ta[:], in0=pw[:, 0:64], scalar1=r[:, 0:1],
                        scalar2=None, op0=AluOpType.mult)
                    tb2 = flp.tile([128, 64], dt.float32, tag="tb2")
                    nc.vector.scalar_tensor_tensor(
                        out=tb2[:], in0=pw[:, 64:128], scalar=r[:, 1:2],
                        in1=ta[:], op0=AluOpType.mult, op1=AluOpType.add)
                    ob2 = flp.tile([128, 64], dt.float32, tag="ob2")
                    nc.vector.scalar_tensor_tensor(
                        out=ob2[:], in0=tb2[:], scalar=0.5,
                        in1=brep_sb[:, 0:64], op0=AluOpType.mult,
                        op1=AluOpType.add)
                    nc.sync.dma_start(outd[w * WIN:(w + 1) * WIN, :],
                                      ob2[0:WIN, :])

            tioff = [0]
            for w in range(nwin):
                tioff.append(tioff[-1] + nA[w] + nB[w])

            def stage_gather(w):
                """T1 row gathers for window w (emitted 2 windows ahead)."""
                ti, g = tioff[w], nA[w] + nB[w]
                T1g = t1p.tile([128, GW, 256], dt.float16, tag="t1g")
                # split each half-gather across SWDGE queues: each queue has
                # its own Q7 cpu pair (parallel descriptor generation) and
                # its own descriptor ring (2048 rows/gather fills a ring).
                subs = []           # (tile_lo, tile_hi, src_view)
                if nA[w]:
                    if NSWQ >= 4 and nA[w] > 1:
                        h1 = (nA[w] + 1) // 2
                        subs += [(0, h1, t1A), (h1, nA[w], t1A)]
                    else:
                        subs += [(0, nA[w], t1A)]
                if nB[w]:
                    if NSWQ >= 4 and nB[w] > 1:
                        h1 = (nB[w] + 1) // 2
                        subs += [(nA[w], nA[w] + h1, t1B),
                                 (nA[w] + h1, g, t1B)]
                    else:
                        subs += [(nA[w], g, t1B)]
                for q, (lo, hi, view) in enumerate(subs):
                    ne = (hi - lo) * TILE
                    nc.gpsimd.dma_gather(
                        T1g[:, lo:hi, :], view,
                        i1_sb[:, (ti + lo) * 8:(ti + hi) * 8], ne, ne, 256,
                        single_packet=False, queue_num=q % NSWQ)
                return T1g

            def stage_onehot(w):
                """One-hots + alpha_dst + ad-matmuls for window w (emitted 1
                window ahead; independent of the gathers)."""
                ti, g = tioff[w], nA[w] + nB[w]
                # alpha_dst of this window, partition-major. 128 rows (not
                # 127): row 127 pairs with the pad one-hot and must be a
                # finite value, not uninitialized SBUF.
                adwin = adp.tile([128, 2], dt.float16, tag="adwin")
                nc.sync.dma_start(adwin[:],
                                  T2_dram[w * WIN:w * WIN + 128, :])
                # one-hot S[e, t, j] = (iota_j == dst_rel)
                Sg = ohp.tile([128, GW, 128], dt.bfloat16, tag="sg")
                io = iota_sb[:]
                io3 = bass.AP(tensor=io.tensor, offset=io.offset,
                              ap=[io.ap[0], [0, g], [1, 128]])
                drs = drel_sb[:, ti:ti + g]
                dr3 = bass.AP(tensor=drs.tensor, offset=drs.offset,
                              ap=[drs.ap[0], [1, g], [0, 128]])
                nc.vector.tensor_tensor(out=Sg[:, 0:g, :], in0=io3,
                                        in1=dr3, op=AluOpType.is_equal)

                # transposed one-hot SgT[j, t, e] = (j == dst_rel[t, e]),
                # from a partition-replicated row-major dst_rel (0-stride
                # DMA), computed IN PLACE over the broadcast tile (halves
                # the SBUF footprint of this stage)
                SgT = ohp.tile([128, GW, 128], dt.float16, tag="sgt")
                dv = drelrmd[ti:ti + g, :]
                dap0 = bass.AP(tensor=dv.tensor, offset=dv.offset,
                               ap=[[0, 128], [TILE, g], [1, TILE]])
                nc.sync.dma_start(SgT[:, 0:g, :], dap0)
                ipv = iotap_sb[:]
                ip3 = bass.AP(tensor=ipv.tensor, offset=ipv.offset,
                              ap=[ipv.ap[0], [0, g], [0, 128]])
                nc.vector.tensor_tensor(out=SgT[:, 0:g, :], in0=ip3,
                                        in1=SgT[:, 0:g, :],
                                        op=AluOpType.is_equal)

                # per-edge alpha_dst via TensorE: ad[e, h] = SgT_t.T @ adwin
                psad = psa.tile([128, GW, 2], dt.float32, tag="psad")
                for t in range(g):
                    nc.tensor.matmul(psad[:, t, :], SgT[:, t, :], adwin[:],
                                     start=True, stop=True)
                adc = work.tile([128, GW, 2], dt.float16, tag="adc")
                nc.scalar.activation(out=adc[:, 0:g, :], in_=psad[:, 0:g, :],
                                     func=AF.Copy)
                return (Sg, adc)

            def stage_main(w, T1g, oh):
                """score -> exp -> messages -> scatter matmuls."""
                Sg, adc = oh
                g = nA[w] + nB[w]
                # compact copy of alpha_src (strided read is slow on DVE)
                asg = work.tile([128, GW, 2], dt.float16, tag="asg")
                nc.scalar.activation(out=asg[:, 0:g, :],
                                     in_=T1g[:, 0:g, 128:130], func=AF.Copy)

                # score = leaky_relu(as + ad); w = exp(score)
                sc = work.tile([128, GW, 2], dt.float32, tag="sc")
                nc.vector.tensor_tensor(
                    out=sc[:, 0:g, :], in0=asg[:, 0:g, :],
                    in1=adc[:, 0:g, :], op=AluOpType.add)
                nc.vector.scalar_tensor_tensor(
                    out=sc[:, 0:g, :], in0=sc[:, 0:g, :], scalar=0.2,
                    in1=sc[:, 0:g, :], op0=AluOpType.mult,
                    op1=AluOpType.max)
                Mg = work.tile([128, GW, 130], dt.bfloat16, tag="mg")
                nc.scalar.activation(
                    out=Mg[:, 0:g, 128:130], in_=sc[:, 0:g, :],
                    func=AF.Exp)
                wb = Mg[:, 0:g, 128:130]
                win1 = bass.AP(tensor=wb.tensor, offset=wb.offset,
                               ap=[wb.ap[0], [130, g], [1, 2], [0, 64]])
                nc.vector.tensor_tensor(
                    out=Mg[:, 0:g, 0:128].rearrange(
                        "p t (h d) -> p t h d", h=2),
                    in0=T1g[:, 0:g, 0:128].rearrange(
                        "p t (h d) -> p t h d", h=2),
                    in1=win1, op=AluOpType.mult)
                pw = psw.tile([128, 130], dt.float32, tag="pw")
                for t in range(g):
                    nc.tensor.matmul(
                        pw[:], Sg[:, t, :], Mg[:, t, :],
                        start=(t == 0), stop=(t == g - 1))
                return pw

            # emission order: plain per-window (measured faster than explicit
            # software-pipelined orders; the tile framework's semaphores plus
            # multi-buffer pools already overlap adjacent windows)
            PIPE = os.environ.get("K_PIPE", "0")
            if PIPE == "0":
                for w in range(nwin):
                    T1g = stage_gather(w)
                    ohw = stage_onehot(w)
                    pw = stage_main(w, T1g, ohw)
                    flush_window(w, pw)
            else:
                gat = {0: stage_gather(0)}
                if nwin > 1:
                    gat[1] = stage_gather(1)
                oh = {0: stage_onehot(0)}
                pending = None
                for w in range(nwin):
                    if w + 2 < nwin:
                        gat[w + 2] = stage_gather(w + 2)
                    if w + 1 < nwin:
                        oh[w + 1] = stage_onehot(w + 1)
                    pw = stage_main(w, gat.pop(w), oh.pop(w))
                    if pending is not None:
                        flush_window(*pending)
                    pending = (w, pw)
                flush_window(*pending)

    nc.compile()
    return nc


# --------------------------------------------------------------------------
# host orchestration
# --------------------------------------------------------------------------

def _pad_T(x16, cols):
    """[N, 128] fp16 -> transposed padded [128, cols]."""
    out = np.zeros((128, cols), f16)
    out[:, :x16.shape[0]] = x16.T
    return out


def _layer_inputs(sched, xfullT, xshardTs, Wm, att_s, att_d, bias, layer):
    Wf = Wm.astype(f16)
    base = {
        "xT": xfullT,
        "W": np.ascontiguousarray(Wf),
        "WT": np.ascontiguousarray(Wf.T),
        "Ase": _expand_att(att_s),
        "Ade": _expand_att(att_d),
        "iota": np.broadcast_to(np.arange(128, dtype=np.float32),
                                (128, 128)).astype(bf16).copy(),
        "iotaP": np.arange(128, dtype=np.float32).reshape(128, 1).astype(f16),
    }
    br = np.zeros((128, 128), np.float32)
    if layer == 1:
        br[:, :] = bias[None, :]
    else:
        br[:, 0:64] = bias[None, :]
    base["brep"] = br
    maps = []
    for c in range(NCORES):
        m = dict(base)
        m["xTs"] = xshardTs[c]
        m["i1"] = sched["i1"][c]
        m["drel"] = sched["drel"][c]
        m["drel_rm"] = sched["drel_rm"][c]
        maps.append(m)
    return maps


def kernel(**inputs):
    global LAST_EXEC_NS, LAST_RESULTS
    LAST_EXEC_NS = []
    LAST_RESULTS = []
    x = np.asarray(inputs["x"], np.float32)
    edge_index = np.asarray(inputs["edge_index"]).astype(np.int64)

    key = hash(edge_index.tobytes())
    if key not in _CACHE:
        sched = _schedule(edge_index)
        nc1 = _build_program(1, sched)
        nc2 = _build_program(2, sched)
        _CACHE.clear()
        _CACHE[key] = (sched, nc1, nc2)
    sched, nc1, nc2 = _CACHE[key]

    trace = bool(os.environ.get("KERNEL_TRACE"))
    trace_kwargs = {}
    if trace:
        _register_ntff_hook()

    def run(nc, maps):
        res = run_bass_kernel_spmd(nc, maps, core_ids=list(range(NCORES)),
                                   trace=trace, **trace_kwargs)
        LAST_EXEC_NS.append(res.exec_time_ns)
        LAST_RESULTS.append(res)
        return res.results

    # ---------------- launch 1
    x16 = x.astype(f16)
    xfullT = _pad_T(x16, NTAB)
    xshardTs = [np.ascontiguousarray(
        _pad_T(x16[c * NPC:(c + 1) * NPC], NSH)) for c in range(NCORES)]
    maps1 = _layer_inputs(sched, xfullT, xshardTs,
                          np.asarray(inputs["W1"]),
                          np.asarray(inputs["att_src1"]),
                          np.asarray(inputs["att_dst1"]),
                          np.asarray(inputs["b1"], np.float32), 1)
    res1 = run(nc1, maps1)
    out1 = np.concatenate([res1[c]["out"][:NPC] for c in range(NCORES)], 0)

    # ---------------- launch 2
    o16 = out1.astype(f16)
    ofullT = _pad_T(o16, NTAB)
    oshardTs = [np.ascontiguousarray(
        _pad_T(o16[c * NPC:(c + 1) * NPC], NSH)) for c in range(NCORES)]
    maps2 = _layer_inputs(sched, ofullT, oshardTs,
                          np.asarray(inputs["W2"]),
                          np.asarray(inputs["att_src2"]),
                          np.asarray(inputs["att_dst2"]),
                          np.asarray(inputs["b2"], np.float32), 2)
    res2 = run(nc2, maps2)
    out2 = np.concatenate([res2[c]["out"][:NPC] for c in range(NCORES)], 0)
    return out2.astype(np.float32)


# revision 58
# speedup vs baseline: 1.1938x; 1.1938x over previous
"""Trainium2 Bass kernel for a 2-layer GAT (nn_GAT_197568496078).

Strategy (8 NeuronCores, SPMD single program):
  - Edges (+self loops) are sharded by DESTINATION node range: core c owns
    dst in [c*6250, (c+1)*6250). Aggregation is then core-local (no
    collectives). The node feature table is replicated (each core builds it
    with small matmuls; the build DMAs are batched 8 iterations per
    dma_start, since the per-dma_start issue cost on the sync sequencer
    dominated the table-build prologue).
  - Per layer, each core builds an HBM table T1[n] = [h(128) | alpha_src(2)]
    (fp16, 512B rows) and a small per-shard table T2[v_local] = alpha_dst(2),
    then streams its edges one dst-WINDOW (127 dst nodes) at a time:
      gather T1 rows by src (dma_gather; two src halves so indices fit
      int16, each half split in two and spread over 4 SWDGE queues: each
      queue has its own Q7 cpu pair, so descriptor generation -- the
      original bottleneck at ~6ns/row on a single queue -- runs 4-way
      parallel, and each queue's private descriptor ring stays under its
      128-descriptor capacity),
      one-hot S[e, t, j] = (iota_j == dst_rel)  (bf16, DVE),
      transposed one-hot SgT[j, t, e] = (j == dst_rel)  (fp16, DVE, from a
        partition-replicated row-major dst_rel loaded by 0-stride DMA),
      per-edge alpha_dst WITHOUT a per-edge gather: ad[e, h] = SgT_t.T @
        adwin on TensorE (adwin = the window's 127 alpha_dst rows,
        partition-major, one tiny contiguous DMA),
      score = leaky_relu(as + ad); w = exp(score)  (bf16 has the range for
        scores in [-16, 16]; the softmax max-shift cancels exactly in the
        reference, so it is skipped),
      M = [h * w | w]  (bf16),
      psum[j, 0:130] += S.T @ M   (TensorE, fp32 PSUM, per 127-dst window)
    flush: out[j] = msg/denom (+bias, relu/mean-heads).
  - Two launches (layer 1, layer 2); the host re-shards layer-1 output
    between them (index-only work).
  - Per-window tile counts are padded to the max over cores so all 8 cores
    run one identical program; all per-core variation lives in data arrays
    (gather indices, dst_rel).
  - 10.3ms -> 2.4-2.6ms total HW time vs the single-queue/per-edge-T2-gather
    baseline; rel err vs the fp32 reference ~1e-3.
"""
import os
import sys
import numpy as np
import ml_dtypes

sys.path.insert(0, "/opt/trn_rl_repo")

import concourse.bacc as bacc   # noqa: E402
import concourse.bass as bass   # noqa: E402
import concourse.mybir as mybir # noqa: E402
import concourse.tile as tile   # noqa: E402
from concourse.alu_op_type import AluOpType          # noqa: E402
from concourse.bass_utils import run_bass_kernel_spmd  # noqa: E402
from concourse.library_config import mlp             # noqa: E402

bf16 = ml_dtypes.bfloat16
f16 = np.float16
dt = mybir.dt
AF = mybir.ActivationFunctionType

N, IN_DIM, HID, HEADS, OUT_DIM, E = 50000, 128, 64, 2, 64, 1600000
NCORES = 8
NPC = N // NCORES            # 6250
WIN = 127                    # dst nodes per window (col 127 = pad trash)
NWIN = -(-NPC // WIN)        # 50
TILE = 128
HALF_LIM = 25024             # src < HALF_LIM -> half A (idx base 0)
BASE_B = HALF_LIM            # half B idx = src - BASE_B in [0, 25023]
NTAB = 391 * 128             # 50048 table rows (N padded to 128)
NSH = 50 * 128               # 6400 shard rows (>= NWIN*WIN = 6350)
OUT_ROWS = NWIN * WIN        # 6350

# module-level memo: preprocessing + compiled programs are reused across calls
_CACHE = {}
LAST_EXEC_NS = []            # exec_time_ns of the launches from the last call
LAST_RESULTS = []            # full BassKernelResults of the last call (trace mode)


def _register_ntff_hook():
    """Provide antenv.axon_hooks (absent in this container) so
    run_bass_kernel_spmd(trace=True) can capture NTFF profiles."""
    import types
    import ctypes
    import contextlib

    if "antenv.axon_hooks" in sys.modules:
        return
    try:
        lib = ctypes.CDLL("/opt/axon/libaxon_pjrt.so")
        lib.axon_start_nrt_profile.argtypes = [
            ctypes.POINTER(ctypes.c_int64), ctypes.c_size_t]
        lib.axon_start_nrt_profile.restype = ctypes.c_int64
        lib.axon_stop_nrt_profile.argtypes = [ctypes.c_char_p]
        lib.axon_stop_nrt_profile.restype = ctypes.c_int64
    except (OSError, AttributeError):
        return

    @contextlib.contextmanager
    def _hook(output_dir, device_ids):
        import jax
        jax.devices()
        if device_ids:
            ids = (ctypes.c_int64 * len(device_ids))(*device_ids)
            rc = lib.axon_start_nrt_profile(ids, len(device_ids))
        else:
            rc = lib.axon_start_nrt_profile(None, 0)
        if rc != 0:
            raise RuntimeError(f"axon_start_nrt_profile rc={rc}")
        try:
            yield
        finally:
            n = lib.axon_stop_nrt_profile(str(output_dir).encode())
            print(f"ntff profile: {n} file(s) -> {output_dir}", file=sys.stderr)

    mod = types.ModuleType("antenv.axon_hooks")
    mod.get_axon_ntff_profile_hook = lambda: _hook
    sys.modules["antenv.axon_hooks"] = mod
    # avoid network uploads during offline trace processing
    import concourse.bass_utils as _bu
    _bu.upload_artifacts = lambda p: str(p)


# --------------------------------------------------------------------------
# host-side graph preprocessing (index-only)
# --------------------------------------------------------------------------

def _schedule(edge_index):
    src = np.concatenate([edge_index[0], np.arange(N)]).astype(np.int64)
    dst = np.concatenate([edge_index[1], np.arange(N)]).astype(np.int64)
    shard = dst // NPC

    # collect per (core, window, half) edge lists
    per = [[None] * NWIN for _ in range(NCORES)]
    for c in range(NCORES):
        m = shard == c
        s, d = src[m], dst[m] - c * NPC
        wi = d // WIN
        for w in range(NWIN):
            wm = wi == w
            ws, wd = s[wm], d[wm] - w * WIN
            a = ws < HALF_LIM
            per[c][w] = ((ws[a], wd[a]), (ws[~a], wd[~a]))

    # uniform tile counts per (window, half) = max over cores
    nA = [max(-(-len(per[c][w][0][0]) // TILE) for c in range(NCORES))
          for w in range(NWIN)]
    nB = [max(-(-len(per[c][w][1][0]) // TILE) for c in range(NCORES))
          for w in range(NWIN)]
    ntot = sum(nA) + sum(nB)

    t1 = np.zeros((NCORES, ntot * TILE), np.int16)
    dr = np.zeros((NCORES, ntot * TILE), np.float32)
    for c in range(NCORES):
        pos = 0
        for w in range(NWIN):
            for half, ntiles in ((0, nA[w]), (1, nB[w])):
                ws, wd = per[c][w][half]
                ne, cap = len(ws), ntiles * TILE
                pad = cap - ne
                base = 0 if half == 0 else BASE_B
                # pad idx = -1: the Q7 ucode trims trailing negative indices,
                # so each core only generates descriptors for its true edges
                if os.environ.get("K_TRIM", "0") == "1":
                    fs = np.concatenate([ws - base, np.full(pad, -1)])
                else:
                    psrc = np.full(pad, ws[-1] if ne else base)
                    fs = np.concatenate([ws, psrc]) - base
                fd = np.concatenate([wd, np.full(pad, WIN)])
                t1[c, pos:pos + cap] = fs.astype(np.int16)
                dr[c, pos:pos + cap] = fd
                pos += cap
        assert pos == ntot * TILE

    def wrap_idx(a):  # -> [128, n/16] wrapped for the 8 Q7 cores
        return np.ascontiguousarray(np.tile(a.reshape(-1, 16).T, (8, 1)))

    i1 = [wrap_idx(t1[c]) for c in range(NCORES)]
    drel = [np.ascontiguousarray(dr[c].reshape(-1, TILE).T.astype(bf16))
            for c in range(NCORES)]
    drel_rm = [np.ascontiguousarray(dr[c].reshape(-1, TILE).astype(f16))
               for c in range(NCORES)]
    return {"nA": nA, "nB": nB, "ntot": ntot, "i1": i1, "drel": drel,
            "drel_rm": drel_rm}


def _expand_att(a):
    """att [heads, dim] -> [128, heads] block-diagonal expansion (layout only)."""
    heads, dim = a.shape
    out = np.zeros((heads * dim, heads), np.float32)
    for h in range(heads):
        out[h * dim:(h + 1) * dim, h] = a[h]
    return out.astype(f16)


# --------------------------------------------------------------------------
# device program (identical for all cores; layer 1/2 differ only in flush)
# --------------------------------------------------------------------------

def _build_program(layer, sched, nwin=NWIN):
    nA, nB, ntot = sched["nA"], sched["nB"], sched["ntot"]
    GW = max(nA[w] + nB[w] for w in range(nwin))
    NSWQ = int(os.environ.get("K_NSWQ", "4"))
    nc = bacc.Bacc("TRN2", target_bir_lowering=False, debug=False,
                   enable_asserts=False, num_devices=NCORES,
                   num_swdge_queues=NSWQ)

    xT = nc.dram_tensor("xT", [128, NTAB], dt.float16, kind="ExternalInput")
    xTs = nc.dram_tensor("xTs", [128, NSH], dt.float16, kind="ExternalInput")
    W = nc.dram_tensor("W", [128, 128], dt.float16, kind="ExternalInput")
    WT = nc.dram_tensor("WT", [128, 128], dt.float16, kind="ExternalInput")
    Ase = nc.dram_tensor("Ase", [128, 2], dt.float16, kind="ExternalInput")
    Ade = nc.dram_tensor("Ade", [128, 2], dt.float16, kind="ExternalInput")
    brep = nc.dram_tensor("brep", [128, 128], dt.float32, kind="ExternalInput")
    iota = nc.dram_tensor("iota", [128, 128], dt.bfloat16, kind="ExternalInput")
    iotaP = nc.dram_tensor("iotaP", [128, 1], dt.float16, kind="ExternalInput")
    i1d = nc.dram_tensor("i1", [128, ntot * 8], dt.int16, kind="ExternalInput")
    dreld = nc.dram_tensor("drel", [128, ntot], dt.bfloat16, kind="ExternalInput")
    drelrmd = nc.dram_tensor("drel_rm", [ntot, TILE], dt.float16,
                             kind="ExternalInput")
    if layer == 1:
        outd = nc.dram_tensor("out", [OUT_ROWS, 128], dt.float16,
                              kind="ExternalOutput")
    else:
        outd = nc.dram_tensor("out", [OUT_ROWS, 64], dt.float32,
                              kind="ExternalOutput")

    with tile.TileContext(nc) as tc:
        with (
            tc.tile_pool(name="const", bufs=1) as constp,
            tc.tile_pool(name="tb", bufs=3) as tbp,
            tc.tile_pool(name="t1p", bufs=3) as t1p,
            tc.tile_pool(name="ohp", bufs=3 if layer == 1 else 2) as ohp,
            tc.tile_pool(name="work", bufs=2) as work,
            tc.tile_pool(name="adp", bufs=3) as adp,
            tc.tile_pool(name="fl", bufs=2) as flp,
            tc.tile_pool(name="pst", bufs=2, space="PSUM") as pst,
            tc.tile_pool(name="psw", bufs=2, space="PSUM") as psw,
            tc.tile_pool(name="psa", bufs=2, space="PSUM") as psa,
            tc.tile_pool(name="dram", bufs=1, space="DRAM") as dram,
        ):
            nc.gpsimd.load_library(mlp)

            T1_dram = dram.tile([NTAB, 256], dt.float16)
            T2_dram = dram.tile([NSH, 2], dt.float16)

            # ---- constants
            iota_sb = constp.tile([128, 128], dt.bfloat16)
            nc.sync.dma_start(iota_sb[:], iota[:])
            iotap_sb = constp.tile([128, 1], dt.float16)
            nc.sync.dma_start(iotap_sb[:], iotaP[:])
            brep_sb = constp.tile([128, 128], dt.float32)
            nc.sync.dma_start(brep_sb[:], brep[:])
            i1_sb = constp.tile([128, ntot * 8], dt.int16)
            nc.sync.dma_start(i1_sb[:], i1d[:])
            drel_sb = constp.tile([128, ntot], dt.bfloat16)
            nc.sync.dma_start(drel_sb[:], dreld[:])

            # ---- weight fold: We = [W | W @ Ase]; wd = W @ Ade
            wt_sb = constp.tile([128, 128], dt.float16)
            nc.sync.dma_start(wt_sb[:], WT[:])
            ase_sb = constp.tile([128, 2], dt.float16)
            nc.sync.dma_start(ase_sb[:], Ase[:])
            ade_sb = constp.tile([128, 2], dt.float16)
            nc.sync.dma_start(ade_sb[:], Ade[:])
            we_sb = constp.tile([128, 130], dt.float16)
            nc.sync.dma_start(we_sb[:, 0:128], W[:])
            wd_sb = constp.tile([128, 2], dt.float16)
            ps = pst.tile([128, 2], dt.float32, tag="pt")
            nc.tensor.matmul(ps[:], wt_sb[:], ase_sb[:])
            nc.scalar.activation(out=we_sb[:, 128:130], in_=ps[:], func=AF.Copy)
            ps2 = pst.tile([128, 2], dt.float32, tag="pt")
            nc.tensor.matmul(ps2[:], wt_sb[:], ade_sb[:])
            nc.scalar.activation(out=wd_sb[:], in_=ps2[:], func=AF.Copy)

            # ---- T2 table build first (windows' alpha_dst depends on it).
            # Batched like the T1 build: one in/out dma_start per 10 blocks
            # (the sync-sequencer issue cost, not bandwidth, is what counts).
            B2 = 10
            for j0 in range(0, NSH // 128, B2):
                nb = min(B2, NSH // 128 - j0)
                xs = tbp.tile([128, B2 * 128], dt.float16, tag="xs")
                nc.sync.dma_start(xs[:, 0:nb * 128],
                                  xTs[:, j0 * 128:(j0 + nb) * 128])
                t2b = tbp.tile([128, B2, 2], dt.float16, tag="t2out")
                for k in range(nb):
                    p2 = pst.tile([128, 2], dt.float32, tag="pt2")
                    nc.tensor.matmul(p2[:], xs[:, k * 128:(k + 1) * 128],
                                     wd_sb[:])
                    nc.scalar.activation(out=t2b[:, k, :], in_=p2[:],
                                         func=AF.Copy)
                d2 = T2_dram[j0 * 128:(j0 + nb) * 128, :]
                d2ap = bass.AP(tensor=d2.tensor, offset=d2.offset,
                               ap=[[2, 128], [128 * 2, nb], [1, 2]])
                nc.sync.dma_start(d2ap, t2b[:, 0:nb, :])

            # ---- T1 table build: [h | alpha_src] for all N.
            # DMAs batched 8 iterations at a time: the per-dma_start issue
            # cost on the sync sequencer (~0.7us) dominated the build.
            B1 = 8

            def build_t1(dst_tile, col0, nrows):
                for j0 in range(0, nrows // 128, B1):
                    nb = min(B1, nrows // 128 - j0)
                    xt = tbp.tile([128, B1 * 128], dt.float16, tag="xt")
                    nc.sync.dma_start(
                        xt[:, 0:nb * 128],
                        xT[:, col0 + j0 * 128:col0 + (j0 + nb) * 128])
                    tb = tbp.tile([128, B1, 130], dt.float16, tag="tbout")
                    for k in range(nb):
                        pt = pst.tile([128, 130], dt.float32, tag="pt")
                        nc.tensor.matmul(pt[:], xt[:, k * 128:(k + 1) * 128],
                                         we_sb[:])
                        nc.scalar.activation(out=tb[:, k, :], in_=pt[:],
                                             func=AF.Copy)
                    dview = dst_tile[j0 * 128:(j0 + nb) * 128, 0:130]
                    dap = bass.AP(tensor=dview.tensor, offset=dview.offset,
                                  ap=[[256, 128], [128 * 256, nb], [1, 130]])
                    nc.sync.dma_start(dap, tb[:, 0:nb, :])

            build_t1(T1_dram, 0, NTAB)

            # ---- edge pipeline
            t1A = T1_dram[0:HALF_LIM, :]
            t1B = T1_dram[BASE_B:NTAB, :]
            def flush_window(w, pw):
                # denom >= exp(LR(-16)) ~ 0.04 for real rows (self loop);
                # +1e-6 keeps the trash/pad rows away from reciprocal(0).
                rd = flp.tile([128, 2], dt.float32, tag="rd")
                nc.vector.tensor_scalar(
                    out=rd[:], in0=pw[:, 128:130], scalar1=1e-6, scalar2=None,
                    op0=AluOpType.add)
                r = flp.tile([128, 2], dt.float32, tag="r")
                nc.vector.reciprocal(r[:], rd[:])
                if layer == 1:
                    f32t = flp.tile([128, 128], dt.float32, tag="f32")
                    for h in range(HEADS):
                        nc.vector.scalar_tensor_tensor(
                            out=f32t[:, h * 64:(h + 1) * 64],
                            in0=pw[:, h * 64:(h + 1) * 64],
                            scalar=r[:, h:h + 1],
                            in1=brep_sb[:, h * 64:(h + 1) * 64],
                            op0=AluOpType.mult, op1=AluOpType.add)
                    ob = flp.tile([128, 128], dt.float16, tag="ob")
                    nc.scalar.activation(out=ob[:], in_=f32t[:], func=AF.Relu)
                    nc.sync.dma_start(outd[w * WIN:(w + 1) * WIN, :],
                                      ob[0:WIN, :])
                else:
                    ta = flp.tile([128, 64], dt.float32, tag="ta")
                    nc.vector.tensor_scalar(
                        out=ta[:], in0=pw[:, 0:64], scalar1=r[:, 0:1],
                        scalar2=None, op0=AluOpType.mult)
                    tb2 = flp.tile([128, 64], dt.float32, tag="tb2")
                    nc.vector.scalar_tensor_tensor(
                        out=tb2[:], in0=pw[:, 64:128], scalar=r[:, 1:2],
                        in1=ta[:], op0=AluOpType.mult, op1=AluOpType.add)
                    ob2 = flp.tile([128, 64], dt.float32, tag="ob2")
                    nc.vector.scalar_tensor_tensor(
                        out=ob2[:], in0=tb2[:], scalar=0.5,
                        in1=brep_sb[:, 0:64], op0=AluOpType.mult,
                        op1=AluOpType.add)
                    nc.sync.dma_start(outd[w * WIN:(w + 1) * WIN, :],
                                      ob2[0:WIN, :])

            tioff = [0]
            for w in range(nwin):
                tioff.append(tioff[-1] + nA[w] + nB[w])

            def stage_gather(w):
                """T1 row gathers for window w (emitted 2 windows ahead)."""
                ti, g = tioff[w], nA[w] + nB[w]
                T1g = t1p.tile([128, GW, 256], dt.float16, tag="t1g")
                # split each half-gather across SWDGE queues: each queue has
                # its own Q7 cpu pair (parallel descriptor generation) and
                # its own descriptor ring (2048 rows/gather fills a ring).
                subs = []           # (tile_lo, tile_hi, src_view)
                if nA[w]:
                    if NSWQ >= 4 and nA[w] > 1:
                        h1 = (nA[w] + 1) // 2
                        subs += [(0, h1, t1A), (h1, nA[w], t1A)]
                    else:
                        subs += [(0, nA[w], t1A)]
                if nB[w]:
                    if NSWQ >= 4 and nB[w] > 1:
                        h1 = (nB[w] + 1) // 2
                        subs += [(nA[w], nA[w] + h1, t1B),
                                 (nA[w] + h1, g, t1B)]
                    else:
                        subs += [(nA[w], g, t1B)]
                for q, (lo, hi, view) in enumerate(subs):
                    ne = (hi - lo) * TILE
                    nc.gpsimd.dma_gather(
                        T1g[:, lo:hi, :], view,
                        i1_sb[:, (ti + lo) * 8:(ti + hi) * 8], ne, ne, 256,
                        single_packet=False, queue_num=q % NSWQ)
                return T1g

            def stage_onehot(w):
                """One-hots + alpha_dst + ad-matmuls for window w (emitted 1
                window ahead; independent of the gathers)."""
                ti, g = tioff[w], nA[w] + nB[w]
                # alpha_dst of this window, partition-major. 128 rows (not
                # 127): row 127 pairs with the pad one-hot and must be a
                # finite value, not uninitialized SBUF.
                adwin = adp.tile([128, 2], dt.float16, tag="adwin")
                nc.sync.dma_start(adwin[:],
                                  T2_dram[w * WIN:w * WIN + 128, :])
                # one-hot S[e, t, j] = (iota_j == dst_rel)
                Sg = ohp.tile([128, GW, 128], dt.bfloat16, tag="sg")
                io = iota_sb[:]
                io3 = bass.AP(tensor=io.tensor, offset=io.offset,
                              ap=[io.ap[0], [0, g], [1, 128]])
                drs = drel_sb[:, ti:ti + g]
                dr3 = bass.AP(tensor=drs.tensor, offset=drs.offset,
                              ap=[drs.ap[0], [1, g], [0, 128]])
                nc.vector.tensor_tensor(out=Sg[:, 0:g, :], in0=io3,
                                        in1=dr3, op=AluOpType.is_equal)

                # transposed one-hot SgT[j, t, e] = (j == dst_rel[t, e]),
                # from a partition-replicated row-major dst_rel (0-stride
                # DMA), computed IN PLACE over the broadcast tile (halves
                # the SBUF footprint of this stage)
                SgT = ohp.tile([128, GW, 128], dt.float16, tag="sgt")
                dv = drelrmd[ti:ti + g, :]
                dap0 = bass.AP(tensor=dv.tensor, offset=dv.offset,
                               ap=[[0, 128], [TILE, g], [1, TILE]])
                nc.sync.dma_start(SgT[:, 0:g, :], dap0)
                ipv = iotap_sb[:]
                ip3 = bass.AP(tensor=ipv.tensor, offset=ipv.offset,
                              ap=[ipv.ap[0], [0, g], [0, 128]])
                nc.vector.tensor_tensor(out=SgT[:, 0:g, :], in0=ip3,
                                        in1=SgT[:, 0:g, :],
                                        op=AluOpType.is_equal)

                # per-edge alpha_dst via TensorE: ad[e, h] = SgT_t.T @ adwin
                psad = psa.tile([128, GW, 2], dt.float32, tag="psad")
                for t in range(g):
                    nc.tensor.matmul(psad[:, t, :], SgT[:, t, :], adwin[:],
                                     start=True, stop=True)
                adc = work.tile([128, GW, 2], dt.float16, tag="adc")
                nc.scalar.activation(out=adc[:, 0:g, :], in_=psad[:, 0:g, :],
                                     func=AF.Copy)
                return (Sg, adc)

            def stage_main(w, T1g, oh):
                """score -> exp -> messages -> scatter matmuls."""
                Sg, adc = oh
                g = nA[w] + nB[w]
                # compact copy of alpha_src (strided read is slow on DVE)
                asg = work.tile([128, GW, 2], dt.float16, tag="asg")
                nc.scalar.activation(out=asg[:, 0:g, :],
                                     in_=T1g[:, 0:g, 128:130], func=AF.Copy)

                # score = leaky_relu(as + ad); w = exp(score)
                sc = work.tile([128, GW, 2], dt.float32, tag="sc")
                nc.vector.tensor_tensor(
                    out=sc[:, 0:g, :], in0=asg[:, 0:g, :],
                    in1=adc[:, 0:g, :], op=AluOpType.add)
                nc.vector.scalar_tensor_tensor(
                    out=sc[:, 0:g, :], in0=sc[:, 0:g, :], scalar=0.2,
                    in1=sc[:, 0:g, :], op0=AluOpType.mult,
                    op1=AluOpType.max)
                Mg = work.tile([128, GW, 130], dt.bfloat16, tag="mg")
                nc.scalar.activation(
                    out=Mg[:, 0:g, 128:130], in_=sc[:, 0:g, :],
                    func=AF.Exp)
                wb = Mg[:, 0:g, 128:130]
                win1 = bass.AP(tensor=wb.tensor, offset=wb.offset,
                               ap=[wb.ap[0], [130, g], [1, 2], [0, 64]])
                nc.vector.tensor_tensor(
                    out=Mg[:, 0:g, 0:128].rearrange(
                        "p t (h d) -> p t h d", h=2),
                    in0=T1g[:, 0:g, 0:128].rearrange(
                        "p t (h d) -> p t h d", h=2),
                    in1=win1, op=AluOpType.mult)
                pw = psw.tile([128, 130], dt.float32, tag="pw")
                for t in range(g):
                    nc.tensor.matmul(
                        pw[:], Sg[:, t, :], Mg[:, t, :],
                        start=(t == 0), stop=(t == g - 1))
                return pw

            # emission order: plain per-window (measured faster than explicit
            # software-pipelined orders; the tile framework's semaphores plus
            # multi-buffer pools already overlap adjacent windows)
            PIPE = os.environ.get("K_PIPE", "0")
            if PIPE == "0":
                for w in range(nwin):
                    T1g = stage_gather(w)
                    ohw = stage_onehot(w)
                    pw = stage_main(w, T1g, ohw)
                    flush_window(w, pw)
            else:
                gat = {0: stage_gather(0)}
                if nwin > 1:
                    gat[1] = stage_gather(1)
                oh = {0: stage_onehot(0)}
                pending = None
                for w in range(nwin):
                    if w + 2 < nwin:
                        gat[w + 2] = stage_gather(w + 2)
                    if w + 1 < nwin:
                        oh[w + 1] = stage_onehot(w + 1)
                    pw = stage_main(w, gat.pop(w), oh.pop(w))
                    if pending is not None:
                        flush_window(*pending)
                    pending = (w, pw)
                flush_window(*pending)

    nc.compile()
    return nc


# --------------------------------------------------------------------------
# host orchestration
# --------------------------------------------------------------------------

def _pad_T(x16, cols):
    """[N, 128] fp16 -> transposed padded [128, cols]."""
    out = np.zeros((128, cols), f16)
    out[:, :x16.shape[0]] = x16.T
    return out


def _layer_inputs(sched, xfullT, xshardTs, Wm, att_s, att_d, bias, layer):
    Wf = Wm.astype(f16)
    base = {
        "xT": xfullT,
        "W": np.ascontiguousarray(Wf),
        "WT": np.ascontiguousarray(Wf.T),
        "Ase": _expand_att(att_s),
        "Ade": _expand_att(att_d),
        "iota": np.broadcast_to(np.arange(128, dtype=np.float32),
                                (128, 128)).astype(bf16).copy(),
        "iotaP": np.arange(128, dtype=np.float32).reshape(128, 1).astype(f16),
    }
    br = np.zeros((128, 128), np.float32)
    if layer == 1:
        br[:, :] = bias[None, :]
    else:
        br[:, 0:64] = bias[None, :]
    base["brep"] = br
    maps = []
    for c in range(NCORES):
        m = dict(base)
        m["xTs"] = xshardTs[c]
        m["i1"] = sched["i1"][c]
        m["drel"] = sched["drel"][c]
        m["drel_rm"] = sched["drel_rm"][c]
        maps.append(m)
    return maps


def kernel(**inputs):
    global LAST_EXEC_NS, LAST_RESULTS
    LAST_EXEC_NS = []
    LAST_RESULTS = []
    x = np.asarray(inputs["x"], np.float32)
    edge_index = np.asarray(inputs["edge_index"]).astype(np.int64)

    key = hash(edge_index.tobytes())
    if key not in _CACHE:
        sched = _schedule(edge_index)
        nc1 = _build_program(1, sched)
        nc2 = _build_program(2, sched)
        _CACHE.clear()
        _CACHE[key] = (sched, nc1, nc2)
    sched, nc1, nc2 = _CACHE[key]

    trace = bool(os.environ.get("KERNEL_TRACE"))
    trace_kwargs = {}
    if trace:
        _register_ntff_hook()

    def run(nc, maps):
        res = run_bass_kernel_spmd(nc, maps, core_ids=list(range(NCORES)),
                                   trace=trace, **trace_kwargs)
        LAST_EXEC_NS.append(res.exec_time_ns)
        LAST_RESULTS.append(res)
        return res.results

    # ---------------- launch 1
    x16 = x.astype(f16)
    xfullT = _pad_T(x16, NTAB)
    xshardTs = [np.ascontiguousarray(
        _pad_T(x16[c * NPC:(c + 1) * NPC], NSH)) for c in range(NCORES)]
    maps1 = _layer_inputs(sched, xfullT, xshardTs,
                          np.asarray(inputs["W1"]),
                          np.asarray(inputs["att_src1"]),
                          np.asarray(inputs["att_dst1"]),
                          np.asarray(inputs["b1"], np.float32), 1)
    res1 = run(nc1, maps1)
    out1 = np.concatenate([res1[c]["out"][:NPC] for c in range(NCORES)], 0)

    # ---------------- launch 2
    o16 = out1.astype(f16)
    ofullT = _pad_T(o16, NTAB)
    oshardTs = [np.ascontiguousarray(
        _pad_T(o16[c * NPC:(c + 1) * NPC], NSH)) for c in range(NCORES)]
    maps2 = _layer_inputs(sched, ofullT, oshardTs,
                          np.asarray(inputs["W2"]),
                          np.asarray(inputs["att_src2"]),
                          np.asarray(inputs["att_dst2"]),
                          np.asarray(inputs["b2"], np.float32), 2)
    res2 = run(nc2, maps2)
    out2 = np.concatenate([res2[c]["out"][:NPC] for c in range(NCORES)], 0)
    return out2.astype(np.float32)
